# revision 37
# baseline (speedup 1.0000x reference)
"""Trainium2 (8 NeuronCores) kernel for a dense causal multi-head attention block.

Problem shapes: B=2, S=2048, D=2048, H=16, DH=128 (fp32 in/out).

Distribution strategy (sharding_hint: tensor-parallel over heads):
  Phase 1 (head parallel): core c owns heads {2c, 2c+1}. It computes, for both
    batches, Q^T/K^T/V^T = W^T @ X^T directly in [DH, S] layout (lhsT = W tile,
    rhs = X^T tile loaded via XBAR DMA-transpose of the bf16 input), then causal
    attention fully on-chip:
       scores^T[k, q] = K^T.T @ Q^T          (PE, one matmul per 128x512 tile)
       p = exp(scores / sqrt(DH))            (ACT, straight from PSUM; no
                                              max-subtraction -- scores are O(1))
       diagonal tiles masked by a 0/1 bf16 mask (DVE)
       z^T[dh, q]  += V_tile.T @ p           (PE, PSUM accumulation over k)
       den[1, q]   += ones.T @ p             (PE rank-reduce for softmax denom)
       z^T *= 1/den (DVE recip + gpsimd partition_broadcast + DVE mult)
  AllToAll (2 MB bf16): reshards z^T from (head-sharded, all rows) to
    (all heads, 512-row shard) so each core owns rows of the final output.
  Phase 2 (row parallel): out[q, d] = Z^T.T @ W_O + b_O for the core's 512 rows.

The host wrapper shards/casts inputs (bf16), runs the SPMD NEFF on cores 0-7,
and concatenates the per-core row slices into the full [2, 2048, 2048] output.
"""

import numpy as np
import ml_dtypes

import concourse.bass as bass
import concourse.mybir as mybir
import concourse.tile as tile
from concourse import bacc
from concourse.bass import ts
from concourse.bass_utils import run_bass_kernel_spmd
from concourse.masks import make_identity

B, S, D, H, DH = 2, 2048, 2048, 16, 128
NCORES = 8
HL = H // NCORES            # heads per core = 2
QB = (B * S) // NCORES      # output rows per core = 512
P = 128
SC = 512                    # free-dim chunk (PSUM bank = 512 fp32)
NSC = S // SC               # 4
NDT = D // P                # 16 contraction tiles for D
NST = S // P                # 16 sequence tiles of 128
NQT = QB // P               # 4 local q tiles in phase 2
NDC = D // SC               # 4 output-dim chunks
SCALE = 1.0 / float(np.sqrt(DH))
MASKW = 384 + SC            # shifted-triangle mask width

F32 = mybir.dt.float32
F32R = mybir.dt.float32r
BF16 = mybir.dt.bfloat16


def build_nc():
    nc = bacc.Bacc("TRN2", target_bir_lowering=False, debug=False,
                   num_devices=NCORES)

    # xt = X^T per batch ([B, D, S]); weights pre-tiled partition-major on the
    # host so every DMA below is contiguous.
    xt = nc.dram_tensor("xt", [B, D, S], BF16, kind="ExternalInput")
    wq = nc.dram_tensor("wq", [HL, P, NDT, DH], BF16, kind="ExternalInput")
    wk = nc.dram_tensor("wk", [HL, P, NDT, DH], BF16, kind="ExternalInput")
    wv = nc.dram_tensor("wv", [HL, P, NDT, DH], BF16, kind="ExternalInput")
    bq = nc.dram_tensor("bq", [DH, HL], F32, kind="ExternalInput")
    bk = nc.dram_tensor("bk", [DH, HL], F32, kind="ExternalInput")
    bv = nc.dram_tensor("bv", [DH, HL], F32, kind="ExternalInput")
    wo = nc.dram_tensor("wo", [H * DH, D], BF16, kind="ExternalInput")
    bo = nc.dram_tensor("bo", [1, D], BF16, kind="ExternalInput")
    out = nc.dram_tensor("out", [QB, D], F32, kind="ExternalOutput")

    Exp = mybir.ActivationFunctionType.Exp
    Ident = mybir.ActivationFunctionType.Identity

    with tile.TileContext(nc) as tc:
        with (
            tc.tile_pool(name="const", bufs=1) as cpool,
            tc.tile_pool(name="dram", bufs=1, space="DRAM") as dpool,
            tc.tile_pool(name="ps_acc", bufs=2, space="PSUM") as ps_acc,
            tc.tile_pool(name="ps_z", bufs=2, space="PSUM") as ps_z,
            tc.tile_pool(name="ps_den", bufs=2, space="PSUM") as ps_den,
        ):
            # ---- constants ----
            ident = cpool.tile([P, P], BF16)
            make_identity(nc, ident)
            ones_col = cpool.tile([P, 1], BF16)
            nc.vector.memset(ones_col, 1.0)
            ones_c32 = cpool.tile([P, 1], F32R)
            nc.vector.tensor_copy(ones_c32, ones_col)
            ones_row = cpool.tile([1, P], BF16)
            nc.vector.memset(ones_row, 1.0)
            # mask[ki, t] = 1.0 iff ki <= t - 384; slices give the 4 shifted
            # causal triangles needed for the diagonal 128x512 tiles.
            mask = cpool.tile([P, MASKW], BF16)
            nc.gpsimd.memset(mask, 1.0)
            nc.gpsimd.affine_select(
                out=mask, in_=mask, compare_op=mybir.AluOpType.is_ge,
                fill=0.0, base=-384, pattern=[[1, MASKW]], channel_multiplier=-1,
            )
            bias_sb = {}
            for nm, t in (("q", bq), ("k", bk), ("v", bv)):
                bb = cpool.tile([P, HL], F32, tag=f"b{nm}")
                nc.sync.dma_start(bb, t.ap())
                bias_sb[nm] = bb
            bo_sb = cpool.tile([1, D], BF16)
            nc.sync.dma_start(bo_sb, bo.ap())

            # one AllToAll per local head index: the first launches halfway
            # through phase 1 and hides under compute of the second head.
            a2a_in = [dpool.tile([NCORES, P, SC], BF16, tag=f"a2a_in{hl}",
                                 name=f"a2a_in{hl}") for hl in range(HL)]
            a2a_out = [dpool.tile([NCORES, P, SC], BF16, tag=f"a2a_out{hl}",
                                  name=f"a2a_out{hl}") for hl in range(HL)]

            with (
                tc.tile_pool(name="wpool", bufs=1) as wpool,
                tc.tile_pool(name="xt", bufs=1) as xtpool,
                tc.tile_pool(name="qkv", bufs=2) as qkvpool,
                tc.tile_pool(name="small", bufs=4) as spool,
            ):
                # per-head weight tiles [d_part, d_tile, dh]
                w_sb = []
                for hl in range(HL):
                    per = []
                    for nm, w in (("wq", wq), ("wk", wk), ("wv", wv)):
                        t_sb = wpool.tile([P, NDT, DH], BF16, tag=f"{nm}{hl}")
                        nc.sync.dma_start(t_sb, w.ap()[hl])
                        per.append(t_sb)
                    w_sb.append(per)

                XT = {}
                for hl in range(HL):
                    for b in range(B):
                        if hl == 0:
                            # X^T for batch b, loaded from the host-
                            # pretransposed input. First s-chunk is its own
                            # tile so the first projection starts after 2MB.
                            xta = xtpool.tile([P, NDT, SC], BF16,
                                              tag=f"xta{b}", name=f"xta{b}")
                            for dt_ in range(NDT):
                                nc.sync.dma_start(
                                    xta[:, dt_, :],
                                    xt.ap()[b][ts(dt_, P), 0:SC])
                            xtb = xtpool.tile([P, NDT, S - SC], BF16,
                                              tag=f"xtb{b}", name=f"xtb{b}")
                            for dt_ in range(NDT):
                                nc.sync.dma_start(
                                    xtb[:, dt_, :],
                                    xt.ap()[b][ts(dt_, P), SC:])
                            XT[b] = (xta, xtb)

                        # ---- projections: Q^T, K^T, V^T in [dh, s] ----
                        QT = qkvpool.tile([P, S], BF16, tag="qt")
                        KT = qkvpool.tile([P, S], BF16, tag="kt")
                        VT = qkvpool.tile([P, S], BF16, tag="vt", bufs=1)
                        for pi, (dst, bcol) in enumerate((
                            (QT, bias_sb["q"]), (KT, bias_sb["k"]),
                            (VT, bias_sb["v"]),
                        )):
                            wt = w_sb[hl][pi]
                            for scp in range(NSC // 2):
                                ps2 = ps_acc.tile([P, 2, SC], F32, tag="acc")
                                for i in range(2):
                                    sc = 2 * scp + i
                                    for dt_ in range(NDT):
                                        rhs = (XT[b][0][:, dt_, :] if sc == 0
                                               else XT[b][1][:, dt_,
                                                             ts(sc - 1, SC)])
                                        nc.tensor.matmul(
                                            ps2[:, i, :], lhsT=wt[:, dt_, :],
                                            rhs=rhs, start=(dt_ == 0),
                                            stop=(dt_ == NDT - 1),
                                            skip_group_check=True)
                                nc.scalar.activation(
                                    dst[:, ts(scp, 2 * SC)],
                                    ps2.rearrange("p a b -> p (a b)"), Ident,
                                    bias=bcol[:, hl:hl + 1], scale=1.0)

                        # ---- V in [k, dh] layout via PE transpose ----
                        V_kd = qkvpool.tile([P, NST, DH], BF16, tag="vkd")
                        for st in range(NST):
                            pst = ps_acc.tile([P, P], BF16, tag="acc")
                            nc.tensor.transpose(pst, VT[:, ts(st, P)], ident)
                            nc.scalar.copy(V_kd[:, st, :], pst)

                        # ---- causal attention (scores pipelined 3 ahead so
                        # PE never stalls on ACT exp / DVE mask). Diagonal
                        # tiles are computed at reduced width: tile j only
                        # has live queries q >= 128*j, and only the first
                        # live 128 columns need the triangular mask. ----
                        for qc in reversed(range(NSC)):
                            z_ps = ps_z.tile([P, SC], F32, tag="z")
                            den_ps = ps_den.tile([1, SC], F32, tag="den")
                            nkt = 4 * qc + 4
                            npair = nkt // 2
                            pexps = {}
                            # exp-sum accumulators: one f32r pair fed by DVE,
                            # one f32 pair fed by GpSimd (merged at the end)
                            dacc_v = spool.tile([P, 2, SC], F32R, tag="daccv",
                                                bufs=1, name="daccv")
                            dacc_g = spool.tile([P, 2, SC], F32, tag="daccg",
                                                bufs=1, name="daccg")

                            def emit_pair(pp, qc=qc, nkt=nkt, pexps=None):
                                # scores+exp for k-tiles (2*pp, 2*pp+1) share
                                # one 2-bank PSUM tile and (when both full)
                                # a single wide exp
                                ps2 = ps_acc.tile([P, 2, SC], F32, tag="acc")
                                pexp2 = spool.tile([P, 2, SC], BF16, tag="p2",
                                                   bufs=3, name="p2")
                                los = []
                                for i in range(2):
                                    kt = 2 * pp + i
                                    j = kt - 4 * qc
                                    lo = 128 * j if j >= 0 else 0
                                    los.append(lo)
                                    nc.tensor.matmul(
                                        ps2[:, i, :SC - lo],
                                        lhsT=KT[:, ts(kt, P)],
                                        rhs=QT[:, qc * SC + lo:(qc + 1) * SC],
                                        start=True, stop=True,
                                        skip_group_check=True)
                                if los == [0, 0]:
                                    nc.scalar.activation(
                                        pexp2, ps2, Exp, bias=0.0, scale=SCALE)
                                else:
                                    for i, lo in enumerate(los):
                                        if lo:
                                            nc.vector.memset(
                                                pexp2[:, i, :lo], 0.0)
                                        nc.scalar.activation(
                                            pexp2[:, i, lo:],
                                            ps2[:, i, :SC - lo], Exp,
                                            bias=0.0, scale=SCALE)
                                for i, lo in enumerate(los):
                                    kt = 2 * pp + i
                                    if kt - 4 * qc >= 0:
                                        nc.vector.tensor_mul(
                                            pexp2[:, i, lo:lo + P],
                                            pexp2[:, i, lo:lo + P],
                                            mask[:, 384:384 + P])
                                # accumulate exp sums; pairs alternate between
                                # the DVE chain and the GpSimd chain
                                if pp % 2 == 0:
                                    if pp < 2:
                                        nc.vector.tensor_copy(dacc_v, pexp2)
                                    else:
                                        nc.vector.tensor_add(
                                            dacc_v, dacc_v, pexp2)
                                else:
                                    if pp < 2:
                                        nc.gpsimd.tensor_copy(dacc_g, pexp2)
                                    else:
                                        nc.gpsimd.tensor_add(
                                            dacc_g, dacc_g, pexp2)
                                pexps[pp] = (pexp2, los)

                            for p0 in range(min(2, npair)):
                                emit_pair(p0, pexps=pexps)
                            for pp in range(npair):
                                if pp + 2 < npair:
                                    emit_pair(pp + 2, pexps=pexps)
                                pexp2, los = pexps.pop(pp)
                                for i, lo in enumerate(los):
                                    kt = 2 * pp + i
                                    nc.tensor.matmul(
                                        z_ps[:, lo:], lhsT=V_kd[:, kt, :],
                                        rhs=pexp2[:, i, lo:],
                                        start=(kt == 0), stop=(kt == nkt - 1),
                                        skip_group_check=True)
                            # merge the GpSimd chain into the DVE chain (f32r)
                            if npair > 1:
                                nc.vector.tensor_add(dacc_v, dacc_v, dacc_g)
                            # denominator: reduce accumulator halves on PE
                            for a in range(2):
                                nc.tensor.matmul(
                                    den_ps, lhsT=ones_c32,
                                    rhs=dacc_v[:, a, :],
                                    start=(a == 0), stop=(a == 1),
                                    skip_group_check=True)
                            # normalize: z^T * (1/den) broadcast over partitions
                            with nc.allow_low_precision(
                                    reason="bf16 softmax denom broadcast"):
                                rden = spool.tile([1, SC], BF16, tag="rden",
                                                  bufs=2)
                                nc.vector.reciprocal(rden, den_ps)
                            rb = spool.tile([P, SC], BF16, tag="rb", bufs=2)
                            nc.gpsimd.partition_broadcast(rb, rden)
                            zs = spool.tile([P, SC], BF16, tag="zs", bufs=2)
                            nc.vector.tensor_mul(zs, z_ps, rb)
                            nc.sync.dma_start(a2a_in[hl][4 * b + qc], zs)

                    # reshard this head's z: all (b, qc) chunks are now queued
                    nc.gpsimd.collective_compute(
                        "AllToAll", mybir.AluOpType.bypass,
                        replica_groups=[list(range(NCORES))],
                        ins=[a2a_in[hl][:]], outs=[a2a_out[hl][:]],
                    )

            # ---- phase 2: output projection for this core's 512 rows ----
            # Split by head parity: even heads (local index 0) arrive with the
            # first AllToAll, so their half of the accumulation overlaps the
            # second collective; odd heads finish and merge.
            with (
                tc.tile_pool(name="p2", bufs=1) as p2pool,
                tc.tile_pool(name="p2o", bufs=2) as p2opool,
            ):
                WO_sb = p2pool.tile([P, H, D], BF16, tag="wo")
                for t in range(H):
                    nc.sync.dma_start(WO_sb[:, t, :], wo.ap()[ts(t, P), :])
                ZT_sb = p2pool.tile([P, H, SC], BF16, tag="zt")
                for j in range(NCORES):
                    nc.sync.dma_start(ZT_sb[:, 2 * j, :], a2a_out[0][j])
                # b_O broadcast over partitions, folded into the even stash
                bo_b = p2pool.tile([P, D], BF16, tag="bo_b")
                nc.gpsimd.partition_broadcast(bo_b, bo_sb)
                part = {}
                for qt in range(NQT):
                    for dc in range(NDC):
                        ops = ps_acc.tile([P, SC], F32, tag="acc")
                        for j in range(NCORES):
                            nc.tensor.matmul(
                                ops, lhsT=ZT_sb[:, 2 * j, ts(qt, P)],
                                rhs=WO_sb[:, 2 * j, ts(dc, SC)],
                                start=(j == 0), stop=(j == NCORES - 1))
                        pt = p2pool.tile([P, SC], F32, tag=f"part{qt}_{dc}",
                                         name=f"part{qt}_{dc}")
                        nc.vector.tensor_add(pt, ops, bo_b[:, ts(dc, SC)])
                        part[qt, dc] = pt
                for j in range(NCORES):
                    nc.sync.dma_start(ZT_sb[:, 2 * j + 1, :], a2a_out[1][j])
                for qt in range(NQT):
                    for dc in range(NDC):
                        ops = ps_acc.tile([P, SC], F32, tag="acc")
                        for j in range(NCORES):
                            nc.tensor.matmul(
                                ops, lhsT=ZT_sb[:, 2 * j + 1, ts(qt, P)],
                                rhs=WO_sb[:, 2 * j + 1, ts(dc, SC)],
                                start=(j == 0), stop=(j == NCORES - 1))
                        osb = p2opool.tile([P, SC], F32, tag="osb")
                        nc.vector.tensor_add(osb, ops, part[qt, dc])
                        nc.sync.dma_start(out.ap()[ts(qt, P), ts(dc, SC)], osb)

    nc.compile()
    return nc


_CACHE = {}


def _get_nc():
    if "nc" not in _CACHE:
        _CACHE["nc"] = build_nc()
    return _CACHE["nc"]


def make_in_maps(resid_pre, W_Q, W_K, W_V, W_O, b_Q, b_K, b_V, b_O):
    bf = ml_dtypes.bfloat16
    x_bf = np.asarray(resid_pre, np.float32).astype(bf)
    xt = np.ascontiguousarray(x_bf.transpose(0, 2, 1))  # [B, D, S]
    # weights pre-tiled to [H, P, NDT, DH]: w_t[h, p, o, k] = W[h, o*P + p, k]
    def tile_w(W):
        Wb = np.asarray(W, np.float32).astype(bf)
        return np.ascontiguousarray(
            Wb.reshape(H, NDT, P, DH).transpose(0, 2, 1, 3))
    WQ, WK, WV = tile_w(W_Q), tile_w(W_K), tile_w(W_V)
    WOf = np.ascontiguousarray(
        np.asarray(W_O, np.float32).reshape(H * DH, D)).astype(bf)
    bQ = np.ascontiguousarray(np.asarray(b_Q, np.float32).T)  # [DH, H]
    bK = np.ascontiguousarray(np.asarray(b_K, np.float32).T)
    bV = np.ascontiguousarray(np.asarray(b_V, np.float32).T)
    bO = np.ascontiguousarray(np.asarray(b_O, np.float32)).reshape(1, D).astype(bf)
    in_maps = []
    for c in range(NCORES):
        hs = slice(c * HL, (c + 1) * HL)
        in_maps.append({
            "xt": xt,
            "wq": np.ascontiguousarray(WQ[hs]),
            "wk": np.ascontiguousarray(WK[hs]),
            "wv": np.ascontiguousarray(WV[hs]),
            "bq": np.ascontiguousarray(bQ[:, hs]),
            "bk": np.ascontiguousarray(bK[:, hs]),
            "bv": np.ascontiguousarray(bV[:, hs]),
            "wo": WOf,
            "bo": bO,
        })
    return in_maps


def assemble(results):
    out = np.empty((B, S, D), np.float32)
    for c in range(NCORES):
        b, r = divmod(c, NCORES // B)  # divmod(c, 4)
        out[b, r * QB:(r + 1) * QB] = results[c]["out"]
    return out


def kernel(resid_pre, W_Q, W_K, W_V, W_O, b_Q, b_K, b_V, b_O,
           _trace=False, _return_raw=False):
    nc = _get_nc()
    in_maps = make_in_maps(resid_pre, W_Q, W_K, W_V, W_O, b_Q, b_K, b_V, b_O)
    res = run_bass_kernel_spmd(nc, in_maps, core_ids=list(range(NCORES)),
                               trace=_trace)
    out = assemble(res.results)
    if _return_raw:
        return out, res
    return out


# revision 38
# speedup vs baseline: 1.3111x; 1.3111x over previous
"""Trainium2 (8 NeuronCores) kernel for a dense causal multi-head attention block.

Problem shapes: B=2, S=2048, D=2048, H=16, DH=128 (fp32 in/out).

Distribution strategy (sharding_hint: tensor-parallel over heads):
  Phase 1 (head parallel): core c owns heads {2c, 2c+1}. It computes, for both
    batches, Q^T/K^T/V^T = W^T @ X^T directly in [DH, S] layout (lhsT = W tile,
    rhs = X^T tile loaded via XBAR DMA-transpose of the bf16 input), then causal
    attention fully on-chip:
       scores^T[k, q] = K^T.T @ Q^T          (PE, one matmul per 128x512 tile)
       p = exp(scores / sqrt(DH))            (ACT, straight from PSUM; no
                                              max-subtraction -- scores are O(1))
       diagonal tiles masked by a 0/1 bf16 mask (DVE)
       z^T[dh, q]  += V_tile.T @ p           (PE, PSUM accumulation over k)
       den[1, q]   += ones.T @ p             (PE rank-reduce for softmax denom)
       z^T *= 1/den (DVE recip + gpsimd partition_broadcast + DVE mult)
  AllToAll (2 MB bf16): reshards z^T from (head-sharded, all rows) to
    (all heads, 512-row shard) so each core owns rows of the final output.
  Phase 2 (row parallel): out[q, d] = Z^T.T @ W_O + b_O for the core's 512 rows.

The host wrapper shards/casts inputs (bf16), runs the SPMD NEFF on cores 0-7,
and concatenates the per-core row slices into the full [2, 2048, 2048] output.
"""

import numpy as np
import ml_dtypes

import concourse.bass as bass
import concourse.mybir as mybir
import concourse.tile as tile
from concourse import bacc
from concourse.bass import ts
from concourse.bass_utils import run_bass_kernel_spmd
from concourse.masks import make_identity

B, S, D, H, DH = 2, 2048, 2048, 16, 128
NCORES = 8
HL = H // NCORES            # heads per core = 2
QB = (B * S) // NCORES      # output rows per core = 512
P = 128
SC = 512                    # free-dim chunk (PSUM bank = 512 fp32)
NSC = S // SC               # 4
NDT = D // P                # 16 contraction tiles for D
NST = S // P                # 16 sequence tiles of 128
NQT = QB // P               # 4 local q tiles in phase 2
NDC = D // SC               # 4 output-dim chunks
SCALE = 1.0 / float(np.sqrt(DH))
MASKW = 384 + SC            # shifted-triangle mask width

F32 = mybir.dt.float32
F32R = mybir.dt.float32r
BF16 = mybir.dt.bfloat16


def build_nc():
    nc = bacc.Bacc("TRN2", target_bir_lowering=False, debug=False,
                   num_devices=NCORES)

    # xt = X^T per batch ([B, D, S]); weights pre-tiled partition-major on the
    # host so every DMA below is contiguous.
    xt = nc.dram_tensor("xt", [B, D, S], BF16, kind="ExternalInput")
    wq = nc.dram_tensor("wq", [HL, P, NDT, DH], BF16, kind="ExternalInput")
    wk = nc.dram_tensor("wk", [HL, P, NDT, DH], BF16, kind="ExternalInput")
    wv = nc.dram_tensor("wv", [HL, P, NDT, DH], BF16, kind="ExternalInput")
    bq = nc.dram_tensor("bq", [DH, HL], F32, kind="ExternalInput")
    bk = nc.dram_tensor("bk", [DH, HL], F32, kind="ExternalInput")
    bv = nc.dram_tensor("bv", [DH, HL], F32, kind="ExternalInput")
    wo = nc.dram_tensor("wo", [H * DH, D], BF16, kind="ExternalInput")
    bo = nc.dram_tensor("bo", [1, D], BF16, kind="ExternalInput")
    out = nc.dram_tensor("out", [QB, D], F32, kind="ExternalOutput")

    Exp = mybir.ActivationFunctionType.Exp
    Ident = mybir.ActivationFunctionType.Identity

    with tile.TileContext(nc) as tc:
        with (
            tc.tile_pool(name="const", bufs=1) as cpool,
            tc.tile_pool(name="dram", bufs=1, space="DRAM") as dpool,
            tc.tile_pool(name="ps_acc", bufs=2, space="PSUM") as ps_acc,
            tc.tile_pool(name="ps_z", bufs=2, space="PSUM") as ps_z,
            tc.tile_pool(name="ps_den", bufs=2, space="PSUM") as ps_den,
        ):
            # ---- constants ----
            ident = cpool.tile([P, P], BF16)
            make_identity(nc, ident)
            ones_col = cpool.tile([P, 1], BF16)
            nc.vector.memset(ones_col, 1.0)
            ones_c32 = cpool.tile([P, 1], F32R)
            nc.vector.tensor_copy(ones_c32, ones_col)
            ones_row = cpool.tile([1, P], BF16)
            nc.vector.memset(ones_row, 1.0)
            # mask[ki, t] = 1.0 iff ki <= t - 384; slices give the 4 shifted
            # causal triangles needed for the diagonal 128x512 tiles.
            mask = cpool.tile([P, MASKW], BF16)
            nc.gpsimd.memset(mask, 1.0)
            nc.gpsimd.affine_select(
                out=mask, in_=mask, compare_op=mybir.AluOpType.is_ge,
                fill=0.0, base=-384, pattern=[[1, MASKW]], channel_multiplier=-1,
            )
            bias_sb = {}
            for nm, t in (("q", bq), ("k", bk), ("v", bv)):
                bb = cpool.tile([P, HL], F32, tag=f"b{nm}")
                nc.sync.dma_start(bb, t.ap())
                bias_sb[nm] = bb
            bo_sb = cpool.tile([1, D], BF16)
            nc.sync.dma_start(bo_sb, bo.ap())

            # one AllToAll per local head index: the first launches halfway
            # through phase 1 and hides under compute of the second head.
            a2a_in = [dpool.tile([NCORES, P, SC], BF16, tag=f"a2a_in{hl}",
                                 name=f"a2a_in{hl}") for hl in range(HL)]
            a2a_out = [dpool.tile([NCORES, P, SC], BF16, tag=f"a2a_out{hl}",
                                  name=f"a2a_out{hl}") for hl in range(HL)]

            with (
                tc.tile_pool(name="wpool", bufs=1) as wpool,
                tc.tile_pool(name="xt", bufs=1) as xtpool,
                tc.tile_pool(name="qkv", bufs=2) as qkvpool,
                tc.tile_pool(name="small", bufs=4) as spool,
            ):
                # per-head weight tiles [d_part, d_tile, dh]
                w_sb = []
                for hl in range(HL):
                    per = []
                    for nm, w in (("wq", wq), ("wk", wk), ("wv", wv)):
                        t_sb = wpool.tile([P, NDT, DH], BF16, tag=f"{nm}{hl}")
                        nc.sync.dma_start(t_sb, w.ap()[hl])
                        per.append(t_sb)
                    w_sb.append(per)

                XT = {}
                for hl in range(HL):
                    for b in range(B):
                        if hl == 0:
                            # X^T for batch b, loaded from the host-
                            # pretransposed input. First s-chunk is its own
                            # tile so the first projection starts after 2MB.
                            xta = xtpool.tile([P, NDT, SC], BF16,
                                              tag=f"xta{b}", name=f"xta{b}")
                            for dt_ in range(NDT):
                                nc.sync.dma_start(
                                    xta[:, dt_, :],
                                    xt.ap()[b][ts(dt_, P), 0:SC])
                            xtb = xtpool.tile([P, NDT, S - SC], BF16,
                                              tag=f"xtb{b}", name=f"xtb{b}")
                            for dt_ in range(NDT):
                                nc.sync.dma_start(
                                    xtb[:, dt_, :],
                                    xt.ap()[b][ts(dt_, P), SC:])
                            XT[b] = (xta, xtb)

                        # ---- projections: Q^T, K^T, V^T in [dh, s] ----
                        QT = qkvpool.tile([P, S], BF16, tag="qt")
                        KT = qkvpool.tile([P, S], BF16, tag="kt")
                        VT = qkvpool.tile([P, S], BF16, tag="vt", bufs=1)
                        for pi, (dst, bcol) in enumerate((
                            (QT, bias_sb["q"]), (KT, bias_sb["k"]),
                            (VT, bias_sb["v"]),
                        )):
                            wt = w_sb[hl][pi]
                            for scp in range(NSC // 2):
                                ps2 = ps_acc.tile([P, 2, SC], F32, tag="acc")
                                for i in range(2):
                                    sc = 2 * scp + i
                                    for dt_ in range(NDT):
                                        rhs = (XT[b][0][:, dt_, :] if sc == 0
                                               else XT[b][1][:, dt_,
                                                             ts(sc - 1, SC)])
                                        nc.tensor.matmul(
                                            ps2[:, i, :], lhsT=wt[:, dt_, :],
                                            rhs=rhs, start=(dt_ == 0),
                                            stop=(dt_ == NDT - 1),
                                            skip_group_check=True)
                                nc.scalar.activation(
                                    dst[:, ts(scp, 2 * SC)],
                                    ps2.rearrange("p a b -> p (a b)"), Ident,
                                    bias=bcol[:, hl:hl + 1], scale=1.0)

                        # ---- V in [k, dh] layout via PE transpose ----
                        V_kd = qkvpool.tile([P, NST, DH], BF16, tag="vkd")
                        for st in range(NST):
                            pst = ps_acc.tile([P, P], BF16, tag="acc")
                            nc.tensor.transpose(pst, VT[:, ts(st, P)], ident)
                            nc.scalar.copy(V_kd[:, st, :], pst)

                        # ---- causal attention (scores pipelined 3 ahead so
                        # PE never stalls on ACT exp / DVE mask). Diagonal
                        # tiles are computed at reduced width: tile j only
                        # has live queries q >= 128*j, and only the first
                        # live 128 columns need the triangular mask. ----
                        for qc in reversed(range(NSC)):
                            z_ps = ps_z.tile([P, SC], F32, tag="z")
                            den_ps = ps_den.tile([1, SC], F32, tag="den")
                            nkt = 4 * qc + 4
                            npair = nkt // 2
                            pexps = {}
                            # exp-sum accumulators: four short bf16 chains
                            # (depth <= 2 adds each) keep the DVE in 2x mode
                            # and off the critical path
                            nch = min(4, npair)
                            dacc = [spool.tile([P, 2, SC], BF16,
                                               tag=f"dac{c}", bufs=1,
                                               name=f"dac{c}")
                                    for c in range(nch)]

                            def emit_pair(pp, qc=qc, nkt=nkt, pexps=None):
                                # scores+exp for k-tiles (2*pp, 2*pp+1) share
                                # one 2-bank PSUM tile and (when both full)
                                # a single wide exp
                                ps2 = ps_acc.tile([P, 2, SC], F32, tag="acc")
                                pexp2 = spool.tile([P, 2, SC], BF16, tag="p2",
                                                   bufs=3, name="p2")
                                los = []
                                for i in range(2):
                                    kt = 2 * pp + i
                                    j = kt - 4 * qc
                                    lo = 128 * j if j >= 0 else 0
                                    los.append(lo)
                                    nc.tensor.matmul(
                                        ps2[:, i, :SC - lo],
                                        lhsT=KT[:, ts(kt, P)],
                                        rhs=QT[:, qc * SC + lo:(qc + 1) * SC],
                                        start=True, stop=True,
                                        skip_group_check=True)
                                if los == [0, 0]:
                                    nc.scalar.activation(
                                        pexp2, ps2, Exp, bias=0.0, scale=SCALE)
                                else:
                                    for i, lo in enumerate(los):
                                        if lo:
                                            nc.vector.memset(
                                                pexp2[:, i, :lo], 0.0)
                                        nc.scalar.activation(
                                            pexp2[:, i, lo:],
                                            ps2[:, i, :SC - lo], Exp,
                                            bias=0.0, scale=SCALE)
                                for i, lo in enumerate(los):
                                    kt = 2 * pp + i
                                    if kt - 4 * qc >= 0:
                                        nc.vector.tensor_mul(
                                            pexp2[:, i, lo:lo + P],
                                            pexp2[:, i, lo:lo + P],
                                            mask[:, 384:384 + P])
                                # accumulate exp sums round-robin over the
                                # short bf16 chains
                                da = dacc[pp % nch]
                                if pp < nch:
                                    nc.vector.tensor_copy(da, pexp2)
                                else:
                                    nc.vector.tensor_add(da, da, pexp2)
                                pexps[pp] = (pexp2, los)

                            for p0 in range(min(2, npair)):
                                emit_pair(p0, pexps=pexps)
                            for pp in range(npair):
                                if pp + 2 < npair:
                                    emit_pair(pp + 2, pexps=pexps)
                                pexp2, los = pexps.pop(pp)
                                for i, lo in enumerate(los):
                                    kt = 2 * pp + i
                                    nc.tensor.matmul(
                                        z_ps[:, lo:], lhsT=V_kd[:, kt, :],
                                        rhs=pexp2[:, i, lo:],
                                        start=(kt == 0), stop=(kt == nkt - 1),
                                        skip_group_check=True)
                            # merge chains pairwise, then reduce on PE
                            if nch == 4:
                                nc.vector.tensor_add(dacc[0], dacc[0], dacc[1])
                                nc.vector.tensor_add(dacc[2], dacc[2], dacc[3])
                                nc.vector.tensor_add(dacc[0], dacc[0], dacc[2])
                            elif nch > 1:
                                for c in range(1, nch):
                                    nc.vector.tensor_add(dacc[0], dacc[0],
                                                         dacc[c])
                            for a in range(2):
                                nc.tensor.matmul(
                                    den_ps, lhsT=ones_col,
                                    rhs=dacc[0][:, a, :],
                                    start=(a == 0), stop=(a == 1),
                                    skip_group_check=True)
                            # normalize: z^T * (1/den) broadcast over partitions
                            with nc.allow_low_precision(
                                    reason="bf16 softmax denom broadcast"):
                                rden = spool.tile([1, SC], BF16, tag="rden",
                                                  bufs=2)
                                nc.vector.reciprocal(rden, den_ps)
                            rb = spool.tile([P, SC], BF16, tag="rb", bufs=2)
                            nc.gpsimd.partition_broadcast(rb, rden)
                            zs = spool.tile([P, SC], BF16, tag="zs", bufs=2)
                            nc.vector.tensor_mul(zs, z_ps, rb)
                            nc.sync.dma_start(a2a_in[hl][4 * b + qc], zs)

                    # reshard this head's z: all (b, qc) chunks are now queued
                    nc.gpsimd.collective_compute(
                        "AllToAll", mybir.AluOpType.bypass,
                        replica_groups=[list(range(NCORES))],
                        ins=[a2a_in[hl][:]], outs=[a2a_out[hl][:]],
                    )

            # ---- phase 2: output projection for this core's 512 rows ----
            # Split by head parity: even heads (local index 0) arrive with the
            # first AllToAll, so their half of the accumulation overlaps the
            # second collective; odd heads finish and merge.
            with (
                tc.tile_pool(name="p2", bufs=1) as p2pool,
                tc.tile_pool(name="p2o", bufs=2) as p2opool,
            ):
                WO_sb = p2pool.tile([P, H, D], BF16, tag="wo")
                for t in range(H):
                    nc.sync.dma_start(WO_sb[:, t, :], wo.ap()[ts(t, P), :])
                ZT_sb = p2pool.tile([P, H, SC], BF16, tag="zt")
                for j in range(NCORES):
                    nc.sync.dma_start(ZT_sb[:, 2 * j, :], a2a_out[0][j])
                # b_O broadcast over partitions, folded into the even stash
                bo_b = p2pool.tile([P, D], BF16, tag="bo_b")
                nc.gpsimd.partition_broadcast(bo_b, bo_sb)
                part = {}
                for qt in range(NQT):
                    for dc in range(NDC):
                        ops = ps_acc.tile([P, SC], F32, tag="acc")
                        for j in range(NCORES):
                            nc.tensor.matmul(
                                ops, lhsT=ZT_sb[:, 2 * j, ts(qt, P)],
                                rhs=WO_sb[:, 2 * j, ts(dc, SC)],
                                start=(j == 0), stop=(j == NCORES - 1))
                        pt = p2pool.tile([P, SC], F32, tag=f"part{qt}_{dc}",
                                         name=f"part{qt}_{dc}")
                        nc.vector.tensor_add(pt, ops, bo_b[:, ts(dc, SC)])
                        part[qt, dc] = pt
                for j in range(NCORES):
                    nc.sync.dma_start(ZT_sb[:, 2 * j + 1, :], a2a_out[1][j])
                for qt in range(NQT):
                    for dc in range(NDC):
                        ops = ps_acc.tile([P, SC], F32, tag="acc")
                        for j in range(NCORES):
                            nc.tensor.matmul(
                                ops, lhsT=ZT_sb[:, 2 * j + 1, ts(qt, P)],
                                rhs=WO_sb[:, 2 * j + 1, ts(dc, SC)],
                                start=(j == 0), stop=(j == NCORES - 1))
                        osb = p2opool.tile([P, SC], F32, tag="osb")
                        nc.vector.tensor_add(osb, ops, part[qt, dc])
                        nc.sync.dma_start(out.ap()[ts(qt, P), ts(dc, SC)], osb)

    nc.compile()
    return nc


_CACHE = {}


def _get_nc():
    if "nc" not in _CACHE:
        _CACHE["nc"] = build_nc()
    return _CACHE["nc"]


def make_in_maps(resid_pre, W_Q, W_K, W_V, W_O, b_Q, b_K, b_V, b_O):
    bf = ml_dtypes.bfloat16
    x_bf = np.asarray(resid_pre, np.float32).astype(bf)
    xt = np.ascontiguousarray(x_bf.transpose(0, 2, 1))  # [B, D, S]
    # weights pre-tiled to [H, P, NDT, DH]: w_t[h, p, o, k] = W[h, o*P + p, k]
    def tile_w(W):
        Wb = np.asarray(W, np.float32).astype(bf)
        return np.ascontiguousarray(
            Wb.reshape(H, NDT, P, DH).transpose(0, 2, 1, 3))
    WQ, WK, WV = tile_w(W_Q), tile_w(W_K), tile_w(W_V)
    WOf = np.ascontiguousarray(
        np.asarray(W_O, np.float32).reshape(H * DH, D)).astype(bf)
    bQ = np.ascontiguousarray(np.asarray(b_Q, np.float32).T)  # [DH, H]
    bK = np.ascontiguousarray(np.asarray(b_K, np.float32).T)
    bV = np.ascontiguousarray(np.asarray(b_V, np.float32).T)
    bO = np.ascontiguousarray(np.asarray(b_O, np.float32)).reshape(1, D).astype(bf)
    in_maps = []
    for c in range(NCORES):
        hs = slice(c * HL, (c + 1) * HL)
        in_maps.append({
            "xt": xt,
            "wq": np.ascontiguousarray(WQ[hs]),
            "wk": np.ascontiguousarray(WK[hs]),
            "wv": np.ascontiguousarray(WV[hs]),
            "bq": np.ascontiguousarray(bQ[:, hs]),
            "bk": np.ascontiguousarray(bK[:, hs]),
            "bv": np.ascontiguousarray(bV[:, hs]),
            "wo": WOf,
            "bo": bO,
        })
    return in_maps


def assemble(results):
    out = np.empty((B, S, D), np.float32)
    for c in range(NCORES):
        b, r = divmod(c, NCORES // B)  # divmod(c, 4)
        out[b, r * QB:(r + 1) * QB] = results[c]["out"]
    return out


def kernel(resid_pre, W_Q, W_K, W_V, W_O, b_Q, b_K, b_V, b_O,
           _trace=False, _return_raw=False):
    nc = _get_nc()
    in_maps = make_in_maps(resid_pre, W_Q, W_K, W_V, W_O, b_Q, b_K, b_V, b_O)
    res = run_bass_kernel_spmd(nc, in_maps, core_ids=list(range(NCORES)),
                               trace=_trace)
    out = assemble(res.results)
    if _return_raw:
        return out, res
    return out


# revision 40
# speedup vs baseline: 1.3674x; 1.0429x over previous
"""Trainium2 (8 NeuronCores) kernel for a dense causal multi-head attention block.

Problem shapes: B=2, S=2048, D=2048, H=16, DH=128 (fp32 in/out).

Distribution strategy (sharding_hint: tensor-parallel over heads):
  Phase 1 (head parallel): core c owns heads {2c, 2c+1}. It computes, for both
    batches, Q^T/K^T/V^T = W^T @ X^T directly in [DH, S] layout (lhsT = W tile,
    rhs = X^T tile loaded via XBAR DMA-transpose of the bf16 input), then causal
    attention fully on-chip:
       scores^T[k, q] = K^T.T @ Q^T          (PE, one matmul per 128x512 tile)
       p = exp(scores / sqrt(DH))            (ACT, straight from PSUM; no
                                              max-subtraction -- scores are O(1))
       diagonal tiles masked by a 0/1 bf16 mask (DVE)
       z^T[dh, q]  += V_tile.T @ p           (PE, PSUM accumulation over k)
       den[1, q]   += ones.T @ p             (PE rank-reduce for softmax denom)
       z^T *= 1/den (DVE recip + gpsimd partition_broadcast + DVE mult)
  AllToAll (2 MB bf16): reshards z^T from (head-sharded, all rows) to
    (all heads, 512-row shard) so each core owns rows of the final output.
  Phase 2 (row parallel): out[q, d] = Z^T.T @ W_O + b_O for the core's 512 rows.

The host wrapper shards/casts inputs (bf16), runs the SPMD NEFF on cores 0-7,
and concatenates the per-core row slices into the full [2, 2048, 2048] output.
"""

import numpy as np
import ml_dtypes

import concourse.bass as bass
import concourse.mybir as mybir
import concourse.tile as tile
from concourse import bacc
from concourse.bass import ts
from concourse.bass_utils import run_bass_kernel_spmd
from concourse.masks import make_identity

B, S, D, H, DH = 2, 2048, 2048, 16, 128
NCORES = 8
HL = H // NCORES            # heads per core = 2
QB = (B * S) // NCORES      # output rows per core = 512
P = 128
SC = 512                    # free-dim chunk (PSUM bank = 512 fp32)
NSC = S // SC               # 4
NDT = D // P                # 16 contraction tiles for D
NST = S // P                # 16 sequence tiles of 128
NQT = QB // P               # 4 local q tiles in phase 2
NDC = D // SC               # 4 output-dim chunks
SCALE = 1.0 / float(np.sqrt(DH))
MASKW = 384 + SC            # shifted-triangle mask width

F32 = mybir.dt.float32
F32R = mybir.dt.float32r
BF16 = mybir.dt.bfloat16


def build_nc():
    nc = bacc.Bacc("TRN2", target_bir_lowering=False, debug=False,
                   num_devices=NCORES)

    # xt = X^T per batch ([B, D, S]); weights pre-tiled partition-major on the
    # host so every DMA below is contiguous.
    xt = nc.dram_tensor("xt", [B, D, S], BF16, kind="ExternalInput")
    wq = nc.dram_tensor("wq", [HL, P, NDT, DH], BF16, kind="ExternalInput")
    wk = nc.dram_tensor("wk", [HL, P, NDT, DH], BF16, kind="ExternalInput")
    wv = nc.dram_tensor("wv", [HL, P, NDT, DH], BF16, kind="ExternalInput")
    bq = nc.dram_tensor("bq", [DH, HL], F32, kind="ExternalInput")
    bk = nc.dram_tensor("bk", [DH, HL], F32, kind="ExternalInput")
    bv = nc.dram_tensor("bv", [DH, HL], F32, kind="ExternalInput")
    wo = nc.dram_tensor("wo", [H * DH, D], BF16, kind="ExternalInput")
    bo = nc.dram_tensor("bo", [1, D], BF16, kind="ExternalInput")
    out = nc.dram_tensor("out", [QB, D], F32, kind="ExternalOutput")

    Exp = mybir.ActivationFunctionType.Exp
    Ident = mybir.ActivationFunctionType.Identity

    with tile.TileContext(nc) as tc:
        with (
            tc.tile_pool(name="const", bufs=1) as cpool,
            tc.tile_pool(name="dram", bufs=1, space="DRAM") as dpool,
            tc.tile_pool(name="ps_acc", bufs=4, space="PSUM") as ps_acc,
            tc.tile_pool(name="ps_z", bufs=2, space="PSUM") as ps_z,
            tc.tile_pool(name="ps_den", bufs=2, space="PSUM") as ps_den,
        ):
            # ---- constants ----
            ident = cpool.tile([P, P], BF16)
            make_identity(nc, ident)
            ones_col = cpool.tile([P, 1], BF16)
            nc.vector.memset(ones_col, 1.0)
            ones_c32 = cpool.tile([P, 1], F32R)
            nc.vector.tensor_copy(ones_c32, ones_col)
            ones_row = cpool.tile([1, P], BF16)
            nc.vector.memset(ones_row, 1.0)
            # mask[ki, t] = 1.0 iff ki <= t - 384; slices give the 4 shifted
            # causal triangles needed for the diagonal 128x512 tiles.
            mask = cpool.tile([P, MASKW], BF16)
            nc.gpsimd.memset(mask, 1.0)
            nc.gpsimd.affine_select(
                out=mask, in_=mask, compare_op=mybir.AluOpType.is_ge,
                fill=0.0, base=-384, pattern=[[1, MASKW]], channel_multiplier=-1,
            )
            bias_sb = {}
            for nm, t in (("q", bq), ("k", bk), ("v", bv)):
                bb = cpool.tile([P, HL], F32, tag=f"b{nm}")
                nc.sync.dma_start(bb, t.ap())
                bias_sb[nm] = bb
            bo_sb = cpool.tile([1, D], BF16)
            nc.sync.dma_start(bo_sb, bo.ap())

            # one AllToAll per local head index: the first launches halfway
            # through phase 1 and hides under compute of the second head.
            a2a_in = [dpool.tile([NCORES, P, SC], BF16, tag=f"a2a_in{hl}",
                                 name=f"a2a_in{hl}") for hl in range(HL)]
            a2a_out = [dpool.tile([NCORES, P, SC], BF16, tag=f"a2a_out{hl}",
                                  name=f"a2a_out{hl}") for hl in range(HL)]

            with (
                tc.tile_pool(name="wpool", bufs=1) as wpool,
                tc.tile_pool(name="xt", bufs=1) as xtpool,
                tc.tile_pool(name="qkv", bufs=2) as qkvpool,
                tc.tile_pool(name="small", bufs=4) as spool,
            ):
                # per-head weight tiles [d_part, d_tile, dh]
                w_sb = []
                for hl in range(HL):
                    per = []
                    for nm, w in (("wq", wq), ("wk", wk), ("wv", wv)):
                        t_sb = wpool.tile([P, NDT, DH], BF16, tag=f"{nm}{hl}")
                        nc.sync.dma_start(t_sb, w.ap()[hl])
                        per.append(t_sb)
                    w_sb.append(per)

                XT = {}
                for hl in range(HL):
                    for b in range(B):
                        if hl == 0:
                            # X^T for batch b, loaded from the host-
                            # pretransposed input. First s-chunk is its own
                            # tile so the first projection starts after 2MB.
                            xta = xtpool.tile([P, NDT, SC], BF16,
                                              tag=f"xta{b}", name=f"xta{b}")
                            for dt_ in range(NDT):
                                nc.sync.dma_start(
                                    xta[:, dt_, :],
                                    xt.ap()[b][ts(dt_, P), 0:SC])
                            xtb = xtpool.tile([P, NDT, S - SC], BF16,
                                              tag=f"xtb{b}", name=f"xtb{b}")
                            for dt_ in range(NDT):
                                nc.sync.dma_start(
                                    xtb[:, dt_, :],
                                    xt.ap()[b][ts(dt_, P), SC:])
                            XT[b] = (xta, xtb)

                        # ---- projections: Q^T, K^T, V^T in [dh, s] ----
                        QT = qkvpool.tile([P, S], BF16, tag="qt")
                        KT = qkvpool.tile([P, S], BF16, tag="kt")
                        VT = qkvpool.tile([P, S], BF16, tag="vt", bufs=1)
                        for pi, (dst, bcol) in enumerate((
                            (QT, bias_sb["q"]), (KT, bias_sb["k"]),
                            (VT, bias_sb["v"]),
                        )):
                            wt = w_sb[hl][pi]
                            for sc in range(NSC):
                                ps = ps_acc.tile([P, SC], F32, tag="acc")
                                for dt_ in range(NDT):
                                    rhs = (XT[b][0][:, dt_, :] if sc == 0 else
                                           XT[b][1][:, dt_, ts(sc - 1, SC)])
                                    nc.tensor.matmul(
                                        ps, lhsT=wt[:, dt_, :], rhs=rhs,
                                        start=(dt_ == 0), stop=(dt_ == NDT - 1))
                                nc.scalar.activation(
                                    dst[:, ts(sc, SC)], ps, Ident,
                                    bias=bcol[:, hl:hl + 1], scale=1.0)

                        # ---- V in [k, dh] layout via PE transpose ----
                        V_kd = qkvpool.tile([P, NST, DH], BF16, tag="vkd")
                        for st in range(NST):
                            pst = ps_acc.tile([P, P], BF16, tag="acc")
                            nc.tensor.transpose(pst, VT[:, ts(st, P)], ident)
                            nc.scalar.copy(V_kd[:, st, :], pst)

                        # ---- causal attention (scores pipelined 3 ahead so
                        # PE never stalls on ACT exp / DVE mask). Diagonal
                        # tiles are computed at reduced width: tile j only
                        # has live queries q >= 128*j, and only the first
                        # live 128 columns need the triangular mask. ----
                        for qc in reversed(range(NSC)):
                            z_ps = ps_z.tile([P, SC], F32, tag="z")
                            den_ps = ps_den.tile([1, SC], F32, tag="den")
                            nkt = 4 * qc + 4
                            pexps = {}
                            # exp-sum accumulators: four short bf16 chains
                            # (depth <= 3 adds each) keep the DVE in 2x mode
                            # and off the critical path
                            nch = min(4, nkt)
                            dacc = [spool.tile([P, SC], BF16,
                                               tag=f"dac{c}", bufs=1,
                                               name=f"dac{c}")
                                    for c in range(nch)]

                            def emit_scores(kt, qc=qc, pexps=None):
                                j = kt - 4 * qc
                                lo = 128 * j if j >= 0 else 0
                                s_ps = ps_acc.tile([P, SC], F32, tag="acc")
                                nc.tensor.matmul(
                                    s_ps[:, :SC - lo], lhsT=KT[:, ts(kt, P)],
                                    rhs=QT[:, qc * SC + lo:(qc + 1) * SC],
                                    start=True, stop=True)
                                pexp = spool.tile([P, SC], BF16, tag="p",
                                                  bufs=5)
                                if lo:
                                    nc.vector.memset(pexp[:, :lo], 0.0)
                                nc.scalar.activation(
                                    pexp[:, lo:], s_ps[:, :SC - lo], Exp,
                                    bias=0.0, scale=SCALE)
                                if j >= 0:
                                    nc.vector.tensor_mul(
                                        pexp[:, lo:lo + P], pexp[:, lo:lo + P],
                                        mask[:, 384:384 + P])
                                da = dacc[kt % nch]
                                if kt < nch:
                                    nc.vector.tensor_copy(da, pexp)
                                else:
                                    nc.vector.tensor_add(da, da, pexp)
                                pexps[kt] = (pexp, lo)

                            for k0 in range(min(3, nkt)):
                                emit_scores(k0, pexps=pexps)
                            for kt in range(nkt):
                                if kt + 3 < nkt:
                                    emit_scores(kt + 3, pexps=pexps)
                                pexp, lo = pexps.pop(kt)
                                nc.tensor.matmul(
                                    z_ps[:, lo:], lhsT=V_kd[:, kt, :],
                                    rhs=pexp[:, lo:],
                                    start=(kt == 0), stop=(kt == nkt - 1),
                                    skip_group_check=True)
                            # merge chains pairwise, then reduce on PE
                            if nch == 4:
                                nc.vector.tensor_add(dacc[0], dacc[0], dacc[1])
                                nc.vector.tensor_add(dacc[2], dacc[2], dacc[3])
                                nc.vector.tensor_add(dacc[0], dacc[0], dacc[2])
                            nc.tensor.matmul(
                                den_ps, lhsT=ones_col, rhs=dacc[0],
                                start=True, stop=True)
                            # normalize: z^T * (1/den) broadcast over partitions
                            with nc.allow_low_precision(
                                    reason="bf16 softmax denom broadcast"):
                                rden = spool.tile([1, SC], BF16, tag="rden",
                                                  bufs=2)
                                nc.vector.reciprocal(rden, den_ps)
                            rb = spool.tile([P, SC], BF16, tag="rb", bufs=2)
                            nc.gpsimd.partition_broadcast(rb, rden)
                            zs = spool.tile([P, SC], BF16, tag="zs", bufs=2)
                            nc.vector.tensor_mul(zs, z_ps, rb)
                            nc.sync.dma_start(a2a_in[hl][4 * b + qc], zs)

                    # reshard this head's z: all (b, qc) chunks are now queued
                    nc.gpsimd.collective_compute(
                        "AllToAll", mybir.AluOpType.bypass,
                        replica_groups=[list(range(NCORES))],
                        ins=[a2a_in[hl][:]], outs=[a2a_out[hl][:]],
                    )

            # ---- phase 2: output projection for this core's 512 rows ----
            # Split by head parity: even heads (local index 0) arrive with the
            # first AllToAll, so their half of the accumulation overlaps the
            # second collective; odd heads finish and merge.
            with (
                tc.tile_pool(name="p2", bufs=1) as p2pool,
                tc.tile_pool(name="p2o", bufs=2) as p2opool,
            ):
                WO_sb = p2pool.tile([P, H, D], BF16, tag="wo")
                for t in range(H):
                    nc.sync.dma_start(WO_sb[:, t, :], wo.ap()[ts(t, P), :])
                ZT_sb = p2pool.tile([P, H, SC], BF16, tag="zt")
                for j in range(NCORES):
                    nc.sync.dma_start(ZT_sb[:, 2 * j, :], a2a_out[0][j])
                # b_O broadcast over partitions, folded into the even stash
                bo_b = p2pool.tile([P, D], BF16, tag="bo_b")
                nc.gpsimd.partition_broadcast(bo_b, bo_sb)
                part = {}
                for qt in range(NQT):
                    for dc in range(NDC):
                        ops = ps_acc.tile([P, SC], F32, tag="acc")
                        for j in range(NCORES):
                            nc.tensor.matmul(
                                ops, lhsT=ZT_sb[:, 2 * j, ts(qt, P)],
                                rhs=WO_sb[:, 2 * j, ts(dc, SC)],
                                start=(j == 0), stop=(j == NCORES - 1))
                        pt = p2pool.tile([P, SC], F32, tag=f"part{qt}_{dc}",
                                         name=f"part{qt}_{dc}")
                        nc.vector.tensor_add(pt, ops, bo_b[:, ts(dc, SC)])
                        part[qt, dc] = pt
                for j in range(NCORES):
                    nc.sync.dma_start(ZT_sb[:, 2 * j + 1, :], a2a_out[1][j])
                for qt in range(NQT):
                    for dc in range(NDC):
                        ops = ps_acc.tile([P, SC], F32, tag="acc")
                        for j in range(NCORES):
                            nc.tensor.matmul(
                                ops, lhsT=ZT_sb[:, 2 * j + 1, ts(qt, P)],
                                rhs=WO_sb[:, 2 * j + 1, ts(dc, SC)],
                                start=(j == 0), stop=(j == NCORES - 1))
                        osb = p2opool.tile([P, SC], F32, tag="osb")
                        nc.vector.tensor_add(osb, ops, part[qt, dc])
                        nc.sync.dma_start(out.ap()[ts(qt, P), ts(dc, SC)], osb)

    nc.compile()
    return nc


_CACHE = {}


def _get_nc():
    if "nc" not in _CACHE:
        _CACHE["nc"] = build_nc()
    return _CACHE["nc"]


def make_in_maps(resid_pre, W_Q, W_K, W_V, W_O, b_Q, b_K, b_V, b_O):
    bf = ml_dtypes.bfloat16
    x_bf = np.asarray(resid_pre, np.float32).astype(bf)
    xt = np.ascontiguousarray(x_bf.transpose(0, 2, 1))  # [B, D, S]
    # weights pre-tiled to [H, P, NDT, DH]: w_t[h, p, o, k] = W[h, o*P + p, k]
    def tile_w(W):
        Wb = np.asarray(W, np.float32).astype(bf)
        return np.ascontiguousarray(
            Wb.reshape(H, NDT, P, DH).transpose(0, 2, 1, 3))
    WQ, WK, WV = tile_w(W_Q), tile_w(W_K), tile_w(W_V)
    WOf = np.ascontiguousarray(
        np.asarray(W_O, np.float32).reshape(H * DH, D)).astype(bf)
    bQ = np.ascontiguousarray(np.asarray(b_Q, np.float32).T)  # [DH, H]
    bK = np.ascontiguousarray(np.asarray(b_K, np.float32).T)
    bV = np.ascontiguousarray(np.asarray(b_V, np.float32).T)
    bO = np.ascontiguousarray(np.asarray(b_O, np.float32)).reshape(1, D).astype(bf)
    in_maps = []
    for c in range(NCORES):
        hs = slice(c * HL, (c + 1) * HL)
        in_maps.append({
            "xt": xt,
            "wq": np.ascontiguousarray(WQ[hs]),
            "wk": np.ascontiguousarray(WK[hs]),
            "wv": np.ascontiguousarray(WV[hs]),
            "bq": np.ascontiguousarray(bQ[:, hs]),
            "bk": np.ascontiguousarray(bK[:, hs]),
            "bv": np.ascontiguousarray(bV[:, hs]),
            "wo": WOf,
            "bo": bO,
        })
    return in_maps


def assemble(results):
    out = np.empty((B, S, D), np.float32)
    for c in range(NCORES):
        b, r = divmod(c, NCORES // B)  # divmod(c, 4)
        out[b, r * QB:(r + 1) * QB] = results[c]["out"]
    return out


def kernel(resid_pre, W_Q, W_K, W_V, W_O, b_Q, b_K, b_V, b_O,
           _trace=False, _return_raw=False):
    nc = _get_nc()
    in_maps = make_in_maps(resid_pre, W_Q, W_K, W_V, W_O, b_Q, b_K, b_V, b_O)
    res = run_bass_kernel_spmd(nc, in_maps, core_ids=list(range(NCORES)),
                               trace=_trace)
    out = assemble(res.results)
    if _return_raw:
        return out, res
    return out


# revision 41
# speedup vs baseline: 1.4093x; 1.0306x over previous
"""Trainium2 (8 NeuronCores) kernel for a dense causal multi-head attention block.

Problem shapes: B=2, S=2048, D=2048, H=16, DH=128 (fp32 in/out).

Distribution strategy (sharding_hint: tensor-parallel over heads):
  Phase 1 (head parallel): core c owns heads {2c, 2c+1}. It computes, for both
    batches, Q^T/K^T/V^T = W^T @ X^T directly in [DH, S] layout (lhsT = W tile,
    rhs = X^T tile loaded via XBAR DMA-transpose of the bf16 input), then causal
    attention fully on-chip:
       scores^T[k, q] = K^T.T @ Q^T          (PE, one matmul per 128x512 tile)
       p = exp(scores / sqrt(DH))            (ACT, straight from PSUM; no
                                              max-subtraction -- scores are O(1))
       diagonal tiles masked by a 0/1 bf16 mask (DVE)
       z^T[dh, q]  += V_tile.T @ p           (PE, PSUM accumulation over k)
       den[1, q]   += ones.T @ p             (PE rank-reduce for softmax denom)
       z^T *= 1/den (DVE recip + gpsimd partition_broadcast + DVE mult)
  AllToAll (2 MB bf16): reshards z^T from (head-sharded, all rows) to
    (all heads, 512-row shard) so each core owns rows of the final output.
  Phase 2 (row parallel): out[q, d] = Z^T.T @ W_O + b_O for the core's 512 rows.

The host wrapper shards/casts inputs (bf16), runs the SPMD NEFF on cores 0-7,
and concatenates the per-core row slices into the full [2, 2048, 2048] output.
"""

import numpy as np
import ml_dtypes

import concourse.bass as bass
import concourse.mybir as mybir
import concourse.tile as tile
from concourse import bacc
from concourse.bass import ts
from concourse.bass_utils import run_bass_kernel_spmd
from concourse.masks import make_identity

B, S, D, H, DH = 2, 2048, 2048, 16, 128
NCORES = 8
HL = H // NCORES            # heads per core = 2
QB = (B * S) // NCORES      # output rows per core = 512
P = 128
SC = 512                    # free-dim chunk (PSUM bank = 512 fp32)
NSC = S // SC               # 4
NDT = D // P                # 16 contraction tiles for D
NST = S // P                # 16 sequence tiles of 128
NQT = QB // P               # 4 local q tiles in phase 2
NDC = D // SC               # 4 output-dim chunks
SCALE = 1.0 / float(np.sqrt(DH))
MASKW = 384 + SC            # shifted-triangle mask width

F32 = mybir.dt.float32
F32R = mybir.dt.float32r
BF16 = mybir.dt.bfloat16


def build_nc():
    nc = bacc.Bacc("TRN2", target_bir_lowering=False, debug=False,
                   num_devices=NCORES)

    # xt = X^T per batch ([B, D, S]); weights pre-tiled partition-major on the
    # host so every DMA below is contiguous.
    xt = nc.dram_tensor("xt", [B, D, S], BF16, kind="ExternalInput")
    wq = nc.dram_tensor("wq", [HL, P, NDT, DH], BF16, kind="ExternalInput")
    wk = nc.dram_tensor("wk", [HL, P, NDT, DH], BF16, kind="ExternalInput")
    wv = nc.dram_tensor("wv", [HL, P, NDT, DH], BF16, kind="ExternalInput")
    bq = nc.dram_tensor("bq", [DH, HL], F32, kind="ExternalInput")
    bk = nc.dram_tensor("bk", [DH, HL], F32, kind="ExternalInput")
    bv = nc.dram_tensor("bv", [DH, HL], F32, kind="ExternalInput")
    wo = nc.dram_tensor("wo", [H * DH, D], BF16, kind="ExternalInput")
    bo = nc.dram_tensor("bo", [1, D], BF16, kind="ExternalInput")
    out = nc.dram_tensor("out", [QB, D], F32, kind="ExternalOutput")

    Exp = mybir.ActivationFunctionType.Exp
    Ident = mybir.ActivationFunctionType.Identity

    with tile.TileContext(nc) as tc:
        with (
            tc.tile_pool(name="const", bufs=1) as cpool,
            tc.tile_pool(name="dram", bufs=1, space="DRAM") as dpool,
            tc.tile_pool(name="ps_acc", bufs=4, space="PSUM") as ps_acc,
            tc.tile_pool(name="ps_z", bufs=2, space="PSUM") as ps_z,
            tc.tile_pool(name="ps_den", bufs=2, space="PSUM") as ps_den,
        ):
            # ---- constants ----
            ident = cpool.tile([P, P], BF16)
            make_identity(nc, ident)
            ones_col = cpool.tile([P, 1], BF16)
            nc.vector.memset(ones_col, 1.0)
            ones_c32 = cpool.tile([P, 1], F32R)
            nc.vector.tensor_copy(ones_c32, ones_col)
            ones_row = cpool.tile([1, P], BF16)
            nc.vector.memset(ones_row, 1.0)
            # mask[ki, t] = 1.0 iff ki <= t - 384; slices give the 4 shifted
            # causal triangles needed for the diagonal 128x512 tiles.
            mask = cpool.tile([P, MASKW], BF16)
            nc.gpsimd.memset(mask, 1.0)
            nc.gpsimd.affine_select(
                out=mask, in_=mask, compare_op=mybir.AluOpType.is_ge,
                fill=0.0, base=-384, pattern=[[1, MASKW]], channel_multiplier=-1,
            )
            bias_sb = {}
            for nm, t in (("q", bq), ("k", bk), ("v", bv)):
                bb = cpool.tile([P, HL], F32, tag=f"b{nm}")
                nc.sync.dma_start(bb, t.ap())
                bias_sb[nm] = bb
            bo_sb = cpool.tile([1, D], BF16)
            nc.sync.dma_start(bo_sb, bo.ap())

            # one AllToAll per local head index: the first launches halfway
            # through phase 1 and hides under compute of the second head.
            a2a_in = [dpool.tile([NCORES, P, SC], BF16, tag=f"a2a_in{hl}",
                                 name=f"a2a_in{hl}") for hl in range(HL)]
            a2a_out = [dpool.tile([NCORES, P, SC], BF16, tag=f"a2a_out{hl}",
                                  name=f"a2a_out{hl}") for hl in range(HL)]

            with (
                tc.tile_pool(name="wpool", bufs=1) as wpool,
                tc.tile_pool(name="xt", bufs=1) as xtpool,
                tc.tile_pool(name="qkv", bufs=2) as qkvpool,
                tc.tile_pool(name="small", bufs=4) as spool,
            ):
                # per-head weight tiles [d_part, d_tile, dh]
                w_sb = []
                for hl in range(HL):
                    per = []
                    for nm, w in (("wq", wq), ("wk", wk), ("wv", wv)):
                        t_sb = wpool.tile([P, NDT, DH], BF16, tag=f"{nm}{hl}")
                        nc.sync.dma_start(t_sb, w.ap()[hl])
                        per.append(t_sb)
                    w_sb.append(per)

                XT = {}
                for hl in range(HL):
                    for b in range(B):
                        if hl == 0:
                            # X^T for batch b, loaded from the host-
                            # pretransposed input. First s-chunk is its own
                            # tile so the first projection starts after 2MB.
                            xta = xtpool.tile([P, NDT, SC], BF16,
                                              tag=f"xta{b}", name=f"xta{b}")
                            for dt_ in range(NDT):
                                nc.sync.dma_start(
                                    xta[:, dt_, :],
                                    xt.ap()[b][ts(dt_, P), 0:SC])
                            xtb = xtpool.tile([P, NDT, S - SC], BF16,
                                              tag=f"xtb{b}", name=f"xtb{b}")
                            for dt_ in range(NDT):
                                nc.sync.dma_start(
                                    xtb[:, dt_, :],
                                    xt.ap()[b][ts(dt_, P), SC:])
                            XT[b] = (xta, xtb)

                        # ---- projections: Q^T, K^T, V^T in [dh, s] ----
                        QT = qkvpool.tile([P, S], BF16, tag="qt")
                        KT = qkvpool.tile([P, S], BF16, tag="kt")
                        VT = qkvpool.tile([P, S], BF16, tag="vt", bufs=1)
                        for pi, (dst, bcol) in enumerate((
                            (QT, bias_sb["q"]), (KT, bias_sb["k"]),
                            (VT, bias_sb["v"]),
                        )):
                            wt = w_sb[hl][pi]
                            for sc in range(NSC):
                                ps = ps_acc.tile([P, SC], F32, tag="acc")
                                for dt_ in range(NDT):
                                    rhs = (XT[b][0][:, dt_, :] if sc == 0 else
                                           XT[b][1][:, dt_, ts(sc - 1, SC)])
                                    nc.tensor.matmul(
                                        ps, lhsT=wt[:, dt_, :], rhs=rhs,
                                        start=(dt_ == 0), stop=(dt_ == NDT - 1))
                                nc.scalar.activation(
                                    dst[:, ts(sc, SC)], ps, Ident,
                                    bias=bcol[:, hl:hl + 1], scale=1.0)

                        # ---- V in [k, dh] layout via PE transpose ----
                        V_kd = qkvpool.tile([P, NST, DH], BF16, tag="vkd")
                        for st in range(NST):
                            pst = ps_acc.tile([P, P], BF16, tag="acc")
                            nc.tensor.transpose(pst, VT[:, ts(st, P)], ident)
                            nc.scalar.copy(V_kd[:, st, :], pst)

                        # ---- causal attention (scores pipelined 3 ahead so
                        # PE never stalls on ACT exp / DVE mask). Diagonal
                        # tiles are computed at reduced width: tile j only
                        # has live queries q >= 128*j, and only the first
                        # live 128 columns need the triangular mask. ----
                        for qc in range(NSC):
                            z_ps = ps_z.tile([P, SC], F32, tag="z")
                            den_ps = ps_den.tile([1, SC], F32, tag="den")
                            nkt = 4 * qc + 4
                            pexps = {}
                            # exp-sum accumulators: four short bf16 chains
                            # (depth <= 3 adds each) keep the DVE in 2x mode
                            # and off the critical path
                            nch = min(4, nkt)
                            dacc = [spool.tile([P, SC], BF16,
                                               tag=f"dac{c}", bufs=1,
                                               name=f"dac{c}")
                                    for c in range(nch)]

                            def emit_scores(kt, qc=qc, pexps=None):
                                j = kt - 4 * qc
                                lo = 128 * j if j >= 0 else 0
                                s_ps = ps_acc.tile([P, SC], F32, tag="acc")
                                nc.tensor.matmul(
                                    s_ps[:, :SC - lo], lhsT=KT[:, ts(kt, P)],
                                    rhs=QT[:, qc * SC + lo:(qc + 1) * SC],
                                    start=True, stop=True)
                                pexp = spool.tile([P, SC], BF16, tag="p",
                                                  bufs=5)
                                nc.scalar.activation(
                                    pexp[:, lo:], s_ps[:, :SC - lo], Exp,
                                    bias=0.0, scale=SCALE)
                                if j >= 0:
                                    nc.vector.tensor_mul(
                                        pexp[:, lo:lo + P], pexp[:, lo:lo + P],
                                        mask[:, 384:384 + P])
                                da = dacc[kt % nch]
                                if kt < nch:
                                    nc.vector.tensor_copy(da[:, lo:],
                                                          pexp[:, lo:])
                                else:
                                    nc.vector.tensor_add(
                                        da[:, lo:], da[:, lo:], pexp[:, lo:])
                                pexps[kt] = (pexp, lo)

                            for k0 in range(min(3, nkt)):
                                emit_scores(k0, pexps=pexps)
                            for kt in range(nkt):
                                if kt + 3 < nkt:
                                    emit_scores(kt + 3, pexps=pexps)
                                pexp, lo = pexps.pop(kt)
                                nc.tensor.matmul(
                                    z_ps[:, lo:], lhsT=V_kd[:, kt, :],
                                    rhs=pexp[:, lo:],
                                    start=(kt == 0), stop=(kt == nkt - 1),
                                    skip_group_check=True)
                            # merge chains pairwise, then reduce on PE.
                            # For qc==0 chain c is only valid from column
                            # 128*c (its tiles are all diagonal).
                            clo = [128 * c if qc == 0 else 0 for c in range(4)]
                            nc.vector.tensor_add(
                                dacc[0][:, clo[1]:], dacc[0][:, clo[1]:],
                                dacc[1][:, clo[1]:])
                            nc.vector.tensor_add(
                                dacc[2][:, clo[3]:], dacc[2][:, clo[3]:],
                                dacc[3][:, clo[3]:])
                            nc.vector.tensor_add(
                                dacc[0][:, clo[2]:], dacc[0][:, clo[2]:],
                                dacc[2][:, clo[2]:])
                            nc.tensor.matmul(
                                den_ps, lhsT=ones_col, rhs=dacc[0],
                                start=True, stop=True)
                            # normalize: z^T * (1/den) broadcast over partitions
                            with nc.allow_low_precision(
                                    reason="bf16 softmax denom broadcast"):
                                rden = spool.tile([1, SC], BF16, tag="rden",
                                                  bufs=2)
                                nc.vector.reciprocal(rden, den_ps)
                            rb = spool.tile([P, SC], BF16, tag="rb", bufs=2)
                            nc.gpsimd.partition_broadcast(rb, rden)
                            zs = spool.tile([P, SC], BF16, tag="zs", bufs=2)
                            nc.vector.tensor_mul(zs, z_ps, rb)
                            nc.sync.dma_start(a2a_in[hl][4 * b + qc], zs)

                    # reshard this head's z: all (b, qc) chunks are now queued
                    nc.gpsimd.collective_compute(
                        "AllToAll", mybir.AluOpType.bypass,
                        replica_groups=[list(range(NCORES))],
                        ins=[a2a_in[hl][:]], outs=[a2a_out[hl][:]],
                    )

            # ---- phase 2: output projection for this core's 512 rows ----
            # Split by head parity: even heads (local index 0) arrive with the
            # first AllToAll, so their half of the accumulation overlaps the
            # second collective; odd heads finish and merge.
            with (
                tc.tile_pool(name="p2", bufs=1) as p2pool,
                tc.tile_pool(name="p2o", bufs=2) as p2opool,
            ):
                WO_sb = p2pool.tile([P, H, D], BF16, tag="wo")
                for t in range(H):
                    nc.sync.dma_start(WO_sb[:, t, :], wo.ap()[ts(t, P), :])
                ZT_sb = p2pool.tile([P, H, SC], BF16, tag="zt")
                for j in range(NCORES):
                    nc.sync.dma_start(ZT_sb[:, 2 * j, :], a2a_out[0][j])
                # b_O broadcast over partitions, folded into the even stash
                bo_b = p2pool.tile([P, D], BF16, tag="bo_b")
                nc.gpsimd.partition_broadcast(bo_b, bo_sb)
                part = {}
                for qt in range(NQT):
                    for dc in range(NDC):
                        ops = ps_acc.tile([P, SC], F32, tag="acc")
                        for j in range(NCORES):
                            nc.tensor.matmul(
                                ops, lhsT=ZT_sb[:, 2 * j, ts(qt, P)],
                                rhs=WO_sb[:, 2 * j, ts(dc, SC)],
                                start=(j == 0), stop=(j == NCORES - 1))
                        pt = p2pool.tile([P, SC], F32, tag=f"part{qt}_{dc}",
                                         name=f"part{qt}_{dc}")
                        nc.vector.tensor_add(pt, ops, bo_b[:, ts(dc, SC)])
                        part[qt, dc] = pt
                for j in range(NCORES):
                    nc.sync.dma_start(ZT_sb[:, 2 * j + 1, :], a2a_out[1][j])
                for qt in range(NQT):
                    for dc in range(NDC):
                        ops = ps_acc.tile([P, SC], F32, tag="acc")
                        for j in range(NCORES):
                            nc.tensor.matmul(
                                ops, lhsT=ZT_sb[:, 2 * j + 1, ts(qt, P)],
                                rhs=WO_sb[:, 2 * j + 1, ts(dc, SC)],
                                start=(j == 0), stop=(j == NCORES - 1))
                        osb = p2opool.tile([P, SC], F32, tag="osb")
                        nc.vector.tensor_add(osb, ops, part[qt, dc])
                        nc.sync.dma_start(out.ap()[ts(qt, P), ts(dc, SC)], osb)

    nc.compile()
    return nc


_CACHE = {}


def _get_nc():
    if "nc" not in _CACHE:
        _CACHE["nc"] = build_nc()
    return _CACHE["nc"]


def make_in_maps(resid_pre, W_Q, W_K, W_V, W_O, b_Q, b_K, b_V, b_O):
    bf = ml_dtypes.bfloat16
    x_bf = np.asarray(resid_pre, np.float32).astype(bf)
    xt = np.ascontiguousarray(x_bf.transpose(0, 2, 1))  # [B, D, S]
    # weights pre-tiled to [H, P, NDT, DH]: w_t[h, p, o, k] = W[h, o*P + p, k]
    def tile_w(W):
        Wb = np.asarray(W, np.float32).astype(bf)
        return np.ascontiguousarray(
            Wb.reshape(H, NDT, P, DH).transpose(0, 2, 1, 3))
    WQ, WK, WV = tile_w(W_Q), tile_w(W_K), tile_w(W_V)
    WOf = np.ascontiguousarray(
        np.asarray(W_O, np.float32).reshape(H * DH, D)).astype(bf)
    bQ = np.ascontiguousarray(np.asarray(b_Q, np.float32).T)  # [DH, H]
    bK = np.ascontiguousarray(np.asarray(b_K, np.float32).T)
    bV = np.ascontiguousarray(np.asarray(b_V, np.float32).T)
    bO = np.ascontiguousarray(np.asarray(b_O, np.float32)).reshape(1, D).astype(bf)
    in_maps = []
    for c in range(NCORES):
        hs = slice(c * HL, (c + 1) * HL)
        in_maps.append({
            "xt": xt,
            "wq": np.ascontiguousarray(WQ[hs]),
            "wk": np.ascontiguousarray(WK[hs]),
            "wv": np.ascontiguousarray(WV[hs]),
            "bq": np.ascontiguousarray(bQ[:, hs]),
            "bk": np.ascontiguousarray(bK[:, hs]),
            "bv": np.ascontiguousarray(bV[:, hs]),
            "wo": WOf,
            "bo": bO,
        })
    return in_maps


def assemble(results):
    out = np.empty((B, S, D), np.float32)
    for c in range(NCORES):
        b, r = divmod(c, NCORES // B)  # divmod(c, 4)
        out[b, r * QB:(r + 1) * QB] = results[c]["out"]
    return out


def kernel(resid_pre, W_Q, W_K, W_V, W_O, b_Q, b_K, b_V, b_O,
           _trace=False, _return_raw=False):
    nc = _get_nc()
    in_maps = make_in_maps(resid_pre, W_Q, W_K, W_V, W_O, b_Q, b_K, b_V, b_O)
    res = run_bass_kernel_spmd(nc, in_maps, core_ids=list(range(NCORES)),
                               trace=_trace)
    out = assemble(res.results)
    if _return_raw:
        return out, res
    return out


# revision 42
# speedup vs baseline: 1.4632x; 1.0383x over previous
"""Trainium2 (8 NeuronCores) kernel for a dense causal multi-head attention block.

Problem shapes: B=2, S=2048, D=2048, H=16, DH=128 (fp32 in/out).

Distribution strategy (sharding_hint: tensor-parallel over heads):
  Phase 1 (head parallel): core c owns heads {2c, 2c+1}. It computes, for both
    batches, Q^T/K^T/V^T = W^T @ X^T directly in [DH, S] layout (lhsT = W tile,
    rhs = X^T tile loaded via XBAR DMA-transpose of the bf16 input), then causal
    attention fully on-chip:
       scores^T[k, q] = K^T.T @ Q^T          (PE, one matmul per 128x512 tile)
       p = exp(scores / sqrt(DH))            (ACT, straight from PSUM; no
                                              max-subtraction -- scores are O(1))
       diagonal tiles masked by a 0/1 bf16 mask (DVE)
       z^T[dh, q]  += V_tile.T @ p           (PE, PSUM accumulation over k)
       den[1, q]   += ones.T @ p             (PE rank-reduce for softmax denom)
       z^T *= 1/den (DVE recip + gpsimd partition_broadcast + DVE mult)
  AllToAll (2 MB bf16): reshards z^T from (head-sharded, all rows) to
    (all heads, 512-row shard) so each core owns rows of the final output.
  Phase 2 (row parallel): out[q, d] = Z^T.T @ W_O + b_O for the core's 512 rows.

The host wrapper shards/casts inputs (bf16), runs the SPMD NEFF on cores 0-7,
and concatenates the per-core row slices into the full [2, 2048, 2048] output.
"""

import numpy as np
import ml_dtypes

import concourse.bass as bass
import concourse.mybir as mybir
import concourse.tile as tile
from concourse import bacc
from concourse.bass import ts
from concourse.bass_utils import run_bass_kernel_spmd
from concourse.masks import make_identity

B, S, D, H, DH = 2, 2048, 2048, 16, 128
NCORES = 8
HL = H // NCORES            # heads per core = 2
QB = (B * S) // NCORES      # output rows per core = 512
P = 128
SC = 512                    # free-dim chunk (PSUM bank = 512 fp32)
NSC = S // SC               # 4
NDT = D // P                # 16 contraction tiles for D
NST = S // P                # 16 sequence tiles of 128
NQT = QB // P               # 4 local q tiles in phase 2
NDC = D // SC               # 4 output-dim chunks
SCALE = 1.0 / float(np.sqrt(DH))
MASKW = 384 + SC            # shifted-triangle mask width

F32 = mybir.dt.float32
F32R = mybir.dt.float32r
BF16 = mybir.dt.bfloat16


def build_nc():
    nc = bacc.Bacc("TRN2", target_bir_lowering=False, debug=False,
                   num_devices=NCORES)

    # xt = X^T per batch ([B, D, S]); weights pre-tiled partition-major on the
    # host so every DMA below is contiguous.
    xt = nc.dram_tensor("xt", [B, D, S], BF16, kind="ExternalInput")
    wq = nc.dram_tensor("wq", [HL, P, NDT, DH], BF16, kind="ExternalInput")
    wk = nc.dram_tensor("wk", [HL, P, NDT, DH], BF16, kind="ExternalInput")
    wv = nc.dram_tensor("wv", [HL, P, NDT, DH], BF16, kind="ExternalInput")
    bq = nc.dram_tensor("bq", [DH, HL], F32, kind="ExternalInput")
    bk = nc.dram_tensor("bk", [DH, HL], F32, kind="ExternalInput")
    bv = nc.dram_tensor("bv", [DH, HL], F32, kind="ExternalInput")
    wo = nc.dram_tensor("wo", [H * DH, D], BF16, kind="ExternalInput")
    bo = nc.dram_tensor("bo", [1, D], BF16, kind="ExternalInput")
    out = nc.dram_tensor("out", [QB, D], F32, kind="ExternalOutput")

    Exp = mybir.ActivationFunctionType.Exp
    Ident = mybir.ActivationFunctionType.Identity

    with tile.TileContext(nc) as tc:
        with (
            tc.tile_pool(name="const", bufs=1) as cpool,
            tc.tile_pool(name="dram", bufs=1, space="DRAM") as dpool,
            tc.tile_pool(name="ps_acc", bufs=5, space="PSUM") as ps_acc,
            tc.tile_pool(name="ps_z", bufs=2, space="PSUM") as ps_z,
            tc.tile_pool(name="ps_den", bufs=1, space="PSUM") as ps_den,
        ):
            # ---- constants ----
            ident = cpool.tile([P, P], BF16)
            make_identity(nc, ident)
            ones_col = cpool.tile([P, 1], BF16)
            nc.vector.memset(ones_col, 1.0)
            ones_c32 = cpool.tile([P, 1], F32R)
            nc.vector.tensor_copy(ones_c32, ones_col)
            ones_row = cpool.tile([1, P], BF16)
            nc.vector.memset(ones_row, 1.0)
            # mask[ki, t] = 1.0 iff ki <= t - 384; slices give the 4 shifted
            # causal triangles needed for the diagonal 128x512 tiles.
            mask = cpool.tile([P, MASKW], BF16)
            nc.gpsimd.memset(mask, 1.0)
            nc.gpsimd.affine_select(
                out=mask, in_=mask, compare_op=mybir.AluOpType.is_ge,
                fill=0.0, base=-384, pattern=[[1, MASKW]], channel_multiplier=-1,
            )
            bias_sb = {}
            for nm, t in (("q", bq), ("k", bk), ("v", bv)):
                bb = cpool.tile([P, HL], F32, tag=f"b{nm}")
                nc.sync.dma_start(bb, t.ap())
                bias_sb[nm] = bb
            bo_sb = cpool.tile([1, D], BF16)
            nc.sync.dma_start(bo_sb, bo.ap())

            # one AllToAll per local head index: the first launches halfway
            # through phase 1 and hides under compute of the second head.
            a2a_in = [dpool.tile([NCORES, P, SC], BF16, tag=f"a2a_in{hl}",
                                 name=f"a2a_in{hl}") for hl in range(HL)]
            a2a_out = [dpool.tile([NCORES, P, SC], BF16, tag=f"a2a_out{hl}",
                                  name=f"a2a_out{hl}") for hl in range(HL)]

            with (
                tc.tile_pool(name="wpool", bufs=1) as wpool,
                tc.tile_pool(name="xt", bufs=1) as xtpool,
                tc.tile_pool(name="qkv", bufs=2) as qkvpool,
                tc.tile_pool(name="small", bufs=4) as spool,
            ):
                # per-head weight tiles [d_part, d_tile, dh]
                w_sb = []
                for hl in range(HL):
                    per = []
                    for nm, w in (("wq", wq), ("wk", wk), ("wv", wv)):
                        t_sb = wpool.tile([P, NDT, DH], BF16, tag=f"{nm}{hl}")
                        nc.sync.dma_start(t_sb, w.ap()[hl])
                        per.append(t_sb)
                    w_sb.append(per)

                XT = {}
                for hl in range(HL):
                    for b in range(B):
                        if hl == 0:
                            # X^T for batch b, loaded from the host-
                            # pretransposed input. First s-chunk is its own
                            # tile so the first projection starts after 2MB.
                            xta = xtpool.tile([P, NDT, SC], BF16,
                                              tag=f"xta{b}", name=f"xta{b}")
                            for dt_ in range(NDT):
                                nc.sync.dma_start(
                                    xta[:, dt_, :],
                                    xt.ap()[b][ts(dt_, P), 0:SC])
                            xtb = xtpool.tile([P, NDT, S - SC], BF16,
                                              tag=f"xtb{b}", name=f"xtb{b}")
                            for dt_ in range(NDT):
                                nc.sync.dma_start(
                                    xtb[:, dt_, :],
                                    xt.ap()[b][ts(dt_, P), SC:])
                            XT[b] = (xta, xtb)

                        # ---- projections: Q^T, K^T, V^T in [dh, s] ----
                        QT = qkvpool.tile([P, S], BF16, tag="qt")
                        KT = qkvpool.tile([P, S], BF16, tag="kt")
                        VT = qkvpool.tile([P, S], BF16, tag="vt", bufs=1)
                        for pi, (dst, bcol) in enumerate((
                            (QT, bias_sb["q"]), (KT, bias_sb["k"]),
                            (VT, bias_sb["v"]),
                        )):
                            wt = w_sb[hl][pi]
                            for sc in range(NSC):
                                ps = ps_acc.tile([P, SC], F32, tag="acc")
                                for dt_ in range(NDT):
                                    rhs = (XT[b][0][:, dt_, :] if sc == 0 else
                                           XT[b][1][:, dt_, ts(sc - 1, SC)])
                                    nc.tensor.matmul(
                                        ps, lhsT=wt[:, dt_, :], rhs=rhs,
                                        start=(dt_ == 0), stop=(dt_ == NDT - 1))
                                nc.scalar.activation(
                                    dst[:, ts(sc, SC)], ps, Ident,
                                    bias=bcol[:, hl:hl + 1], scale=1.0)

                        # ---- V in [k, dh] layout via PE transpose ----
                        V_kd = qkvpool.tile([P, NST, DH], BF16, tag="vkd")
                        for st in range(NST):
                            pst = ps_acc.tile([P, P], BF16, tag="acc")
                            nc.tensor.transpose(pst, VT[:, ts(st, P)], ident)
                            nc.scalar.copy(V_kd[:, st, :], pst)

                        # ---- causal attention (scores pipelined 3 ahead so
                        # PE never stalls on ACT exp / DVE mask). Diagonal
                        # tiles are computed at reduced width: tile j only
                        # has live queries q >= 128*j, and only the first
                        # live 128 columns need the triangular mask. ----
                        for qc in range(NSC):
                            z_ps = ps_z.tile([P, SC], F32, tag="z")
                            den_ps = ps_den.tile([1, SC], F32, tag="den")
                            nkt = 4 * qc + 4
                            pexps = {}
                            # exp-sum accumulators: four short bf16 chains
                            # (depth <= 3 adds each) keep the DVE in 2x mode
                            # and off the critical path
                            nch = min(4, nkt)
                            dacc = [spool.tile([P, SC], BF16,
                                               tag=f"dac{c}", bufs=1,
                                               name=f"dac{c}")
                                    for c in range(nch)]

                            def emit_scores(kt, qc=qc, pexps=None):
                                j = kt - 4 * qc
                                lo = 128 * j if j >= 0 else 0
                                s_ps = ps_acc.tile([P, SC], F32, tag="acc")
                                nc.tensor.matmul(
                                    s_ps[:, :SC - lo], lhsT=KT[:, ts(kt, P)],
                                    rhs=QT[:, qc * SC + lo:(qc + 1) * SC],
                                    start=True, stop=True)
                                pexp = spool.tile([P, SC], BF16, tag="p",
                                                  bufs=5)
                                nc.scalar.activation(
                                    pexp[:, lo:], s_ps[:, :SC - lo], Exp,
                                    bias=0.0, scale=SCALE)
                                if j >= 0:
                                    nc.vector.tensor_mul(
                                        pexp[:, lo:lo + P], pexp[:, lo:lo + P],
                                        mask[:, 384:384 + P])
                                da = dacc[kt % nch]
                                if kt < nch:
                                    nc.vector.tensor_copy(da[:, lo:],
                                                          pexp[:, lo:])
                                else:
                                    nc.vector.tensor_add(
                                        da[:, lo:], da[:, lo:], pexp[:, lo:])
                                pexps[kt] = (pexp, lo)

                            for k0 in range(min(4, nkt)):
                                emit_scores(k0, pexps=pexps)
                            for kt in range(nkt):
                                if kt + 4 < nkt:
                                    emit_scores(kt + 4, pexps=pexps)
                                pexp, lo = pexps.pop(kt)
                                nc.tensor.matmul(
                                    z_ps[:, lo:], lhsT=V_kd[:, kt, :],
                                    rhs=pexp[:, lo:],
                                    start=(kt == 0), stop=(kt == nkt - 1),
                                    skip_group_check=True)
                            # merge chains pairwise, then reduce on PE.
                            # For qc==0 chain c is only valid from column
                            # 128*c (its tiles are all diagonal).
                            clo = [128 * c if qc == 0 else 0 for c in range(4)]
                            nc.vector.tensor_add(
                                dacc[0][:, clo[1]:], dacc[0][:, clo[1]:],
                                dacc[1][:, clo[1]:])
                            nc.vector.tensor_add(
                                dacc[2][:, clo[3]:], dacc[2][:, clo[3]:],
                                dacc[3][:, clo[3]:])
                            nc.vector.tensor_add(
                                dacc[0][:, clo[2]:], dacc[0][:, clo[2]:],
                                dacc[2][:, clo[2]:])
                            nc.tensor.matmul(
                                den_ps, lhsT=ones_col, rhs=dacc[0],
                                start=True, stop=True)
                            # normalize: z^T * (1/den) broadcast over partitions
                            with nc.allow_low_precision(
                                    reason="bf16 softmax denom broadcast"):
                                rden = spool.tile([1, SC], BF16, tag="rden",
                                                  bufs=2)
                                nc.vector.reciprocal(rden, den_ps)
                            rb = spool.tile([P, SC], BF16, tag="rb", bufs=2)
                            nc.gpsimd.partition_broadcast(rb, rden)
                            zs = spool.tile([P, SC], BF16, tag="zs", bufs=2)
                            nc.vector.tensor_mul(zs, z_ps, rb)
                            nc.sync.dma_start(a2a_in[hl][4 * b + qc], zs)

                    # reshard this head's z: all (b, qc) chunks are now queued
                    nc.gpsimd.collective_compute(
                        "AllToAll", mybir.AluOpType.bypass,
                        replica_groups=[list(range(NCORES))],
                        ins=[a2a_in[hl][:]], outs=[a2a_out[hl][:]],
                    )

            # ---- phase 2: output projection for this core's 512 rows ----
            # Split by head parity: even heads (local index 0) arrive with the
            # first AllToAll, so their half of the accumulation overlaps the
            # second collective; odd heads finish and merge.
            with (
                tc.tile_pool(name="p2", bufs=1) as p2pool,
                tc.tile_pool(name="p2o", bufs=2) as p2opool,
            ):
                WO_sb = p2pool.tile([P, H, D], BF16, tag="wo")
                for t in range(H):
                    nc.sync.dma_start(WO_sb[:, t, :], wo.ap()[ts(t, P), :])
                ZT_sb = p2pool.tile([P, H, SC], BF16, tag="zt")
                for j in range(NCORES):
                    nc.sync.dma_start(ZT_sb[:, 2 * j, :], a2a_out[0][j])
                # b_O broadcast over partitions, folded into the even stash
                bo_b = p2pool.tile([P, D], BF16, tag="bo_b")
                nc.gpsimd.partition_broadcast(bo_b, bo_sb)
                part = {}
                for qt in range(NQT):
                    for dc in range(NDC):
                        ops = ps_acc.tile([P, SC], F32, tag="acc")
                        for j in range(NCORES):
                            nc.tensor.matmul(
                                ops, lhsT=ZT_sb[:, 2 * j, ts(qt, P)],
                                rhs=WO_sb[:, 2 * j, ts(dc, SC)],
                                start=(j == 0), stop=(j == NCORES - 1))
                        pt = p2pool.tile([P, SC], F32, tag=f"part{qt}_{dc}",
                                         name=f"part{qt}_{dc}")
                        nc.vector.tensor_add(pt, ops, bo_b[:, ts(dc, SC)])
                        part[qt, dc] = pt
                for j in range(NCORES):
                    nc.sync.dma_start(ZT_sb[:, 2 * j + 1, :], a2a_out[1][j])
                for qt in range(NQT):
                    for dc in range(NDC):
                        ops = ps_acc.tile([P, SC], F32, tag="acc")
                        for j in range(NCORES):
                            nc.tensor.matmul(
                                ops, lhsT=ZT_sb[:, 2 * j + 1, ts(qt, P)],
                                rhs=WO_sb[:, 2 * j + 1, ts(dc, SC)],
                                start=(j == 0), stop=(j == NCORES - 1))
                        osb = p2opool.tile([P, SC], F32, tag="osb")
                        nc.vector.tensor_add(osb, ops, part[qt, dc])
                        nc.sync.dma_start(out.ap()[ts(qt, P), ts(dc, SC)], osb)

    nc.compile()
    return nc


_CACHE = {}


def _get_nc():
    if "nc" not in _CACHE:
        _CACHE["nc"] = build_nc()
    return _CACHE["nc"]


def make_in_maps(resid_pre, W_Q, W_K, W_V, W_O, b_Q, b_K, b_V, b_O):
    bf = ml_dtypes.bfloat16
    x_bf = np.asarray(resid_pre, np.float32).astype(bf)
    xt = np.ascontiguousarray(x_bf.transpose(0, 2, 1))  # [B, D, S]
    # weights pre-tiled to [H, P, NDT, DH]: w_t[h, p, o, k] = W[h, o*P + p, k]
    def tile_w(W):
        Wb = np.asarray(W, np.float32).astype(bf)
        return np.ascontiguousarray(
            Wb.reshape(H, NDT, P, DH).transpose(0, 2, 1, 3))
    WQ, WK, WV = tile_w(W_Q), tile_w(W_K), tile_w(W_V)
    WOf = np.ascontiguousarray(
        np.asarray(W_O, np.float32).reshape(H * DH, D)).astype(bf)
    bQ = np.ascontiguousarray(np.asarray(b_Q, np.float32).T)  # [DH, H]
    bK = np.ascontiguousarray(np.asarray(b_K, np.float32).T)
    bV = np.ascontiguousarray(np.asarray(b_V, np.float32).T)
    bO = np.ascontiguousarray(np.asarray(b_O, np.float32)).reshape(1, D).astype(bf)
    in_maps = []
    for c in range(NCORES):
        hs = slice(c * HL, (c + 1) * HL)
        in_maps.append({
            "xt": xt,
            "wq": np.ascontiguousarray(WQ[hs]),
            "wk": np.ascontiguousarray(WK[hs]),
            "wv": np.ascontiguousarray(WV[hs]),
            "bq": np.ascontiguousarray(bQ[:, hs]),
            "bk": np.ascontiguousarray(bK[:, hs]),
            "bv": np.ascontiguousarray(bV[:, hs]),
            "wo": WOf,
            "bo": bO,
        })
    return in_maps


def assemble(results):
    out = np.empty((B, S, D), np.float32)
    for c in range(NCORES):
        b, r = divmod(c, NCORES // B)  # divmod(c, 4)
        out[b, r * QB:(r + 1) * QB] = results[c]["out"]
    return out


def kernel(resid_pre, W_Q, W_K, W_V, W_O, b_Q, b_K, b_V, b_O,
           _trace=False, _return_raw=False):
    nc = _get_nc()
    in_maps = make_in_maps(resid_pre, W_Q, W_K, W_V, W_O, b_Q, b_K, b_V, b_O)
    res = run_bass_kernel_spmd(nc, in_maps, core_ids=list(range(NCORES)),
                               trace=_trace)
    out = assemble(res.results)
    if _return_raw:
        return out, res
    return out
